# revision 15
# baseline (speedup 1.0000x reference)
"""Trainium2 Bass kernel for ensemble Conv2d (VALID, 3x3).

Problem: out[e,b,j,y,x] = sum_{i,kh,kw} features[e,b,i,y+kh,x+kw] * weight[e,i,j,kh,kw] + bias[e,j]
  features: (4, 32, 64, 64, 64) f32, weight: (4, 64, 128, 3, 3) f32, bias: (4, 128) f32
  output:   (4, 32, 128, 62, 62) f32

Sharding: E*B = 128 images over 8 cores -> each core handles one ensemble
member e = core//2 and 16 of its 32 images. No cross-core communication.

Per-core kernel: implicit-GEMM convolution. For each image, each 3x3 tap
(kh,kw) is one matmul contraction over C_in=64:
  psum[j, (y,x)] += W[:,j,kh,kw].T @ X[:, y+kh, x+kw]
fp16 matmuls (1 col/cycle, fp32 PSUM accumulation; rel err ~3e-4). Two
images are processed concurrently on the two PE-array row halves
(tile_position (0,0) and (64,0)) so the K=64 contraction doesn't waste half
the array; their weight copies live on the matching partition halves.

Schedule notes (from NTFF traces):
- steady-state MM pair slot is ~211ns (496 cols @2.4GHz + stagger) --
  LDWEIGHTS is hidden by the PE reorder window, so the stream is already
  near the PE floor; the losses were head (weight DMA queued behind
  ACT_TABLE_LOAD on the scalar ring), eviction backlog (all 128 PSUM
  evictions on the one scalar engine -> start-matmuls stall on PSUM
  recycle and a ~13us tail drains after the last MM), and f32 stores.
- fixes: weights load via sync+vector rings, evictions alternate
  scalar/vector engines, outputs stored as fp16 (upconverted on host).
"""

import ml_dtypes
import numpy as np

import concourse.bass as bass
import concourse.mybir as mybir
import concourse.tile as tile
from concourse import bacc
from concourse.bass_utils import run_bass_kernel_spmd

E, B, C_IN, H, W = 4, 32, 64, 64, 64
C_OUT, KH, KW = 128, 3, 3
HO, WO = H - KH + 1, W - KW + 1  # 62, 62
N_CORES = 8
B_PER_CORE = (E * B) // N_CORES  # 16
PAIRS = B_PER_CORE // 2  # 8
ROW_BLOCK = 8  # output rows per matmul block: N = 8*62 = 496 <= 512 (one PSUM bank)
ROW_BLOCKS = [(r * ROW_BLOCK, min(ROW_BLOCK, HO - r * ROW_BLOCK))
              for r in range((HO + ROW_BLOCK - 1) // ROW_BLOCK)]
F32 = mybir.dt.float32
BF16 = mybir.dt.bfloat16
FP16 = mybir.dt.float16

MM_DT = FP16
OUT_DT = FP16  # fp16 stores halve output HBM traffic; host upconverts

_CACHE: dict = {}


def _build():
    nc = bacc.Bacc("TRN2", target_bir_lowering=False, debug=False,
                   num_devices=N_CORES)
    x_d = nc.dram_tensor("x", [B_PER_CORE * C_IN, H * W], MM_DT,
                         kind="ExternalInput").ap()
    w_d = nc.dram_tensor("w", [128, KH * KW * C_OUT], MM_DT,
                         kind="ExternalInput").ap()
    b_d = nc.dram_tensor("bias", [C_OUT, 1], F32, kind="ExternalInput").ap()
    y_d = nc.dram_tensor("y", [B_PER_CORE * C_OUT, HO * WO], OUT_DT,
                         kind="ExternalOutput").ap()

    with tile.TileContext(nc) as tc:
        with (
            tc.tile_pool(name="wpool", bufs=1) as wpool,
            tc.tile_pool(name="xpool", bufs=8) as xpool,
            # deep opool: a store's completion semaphore lands ~2.5us after
            # the store issues, so evictions need >4 output bufs per engine
            # path or they stall on store completions (and start-matmuls on
            # PSUM recycle behind them).
            tc.tile_pool(name="opool", bufs=16) as opool,
            tc.tile_pool(name="psum", bufs=8, space=bass.MemorySpace.PSUM) as psum,
        ):
            # HAM warm-up: the PE clock-gate opens only after ~3.4us of
            # sustained PE activity. Input DMAs gate the real stream until
            # ~11us, so burn the wait on dummy matmuls over a memset tile --
            # the real stream then starts at the full 2.4 GHz.
            warm_sb = wpool.tile([128, 512], MM_DT)
            nc.vector.memset(warm_sb[:, :], 0)
            # borrow a main-pool PSUM bank; it recycles into the block
            # pipeline well after the warm-up matmuls retire
            ps_warm = psum.tile([128, 496], F32, tag="ps", name="ps_warm")
            for _ in range(8):
                nc.tensor.matmul(ps_warm[:, :], warm_sb[:, 0:128],
                                 warm_sb[:, 0:496], start=True, stop=True)
            # Weights in 3 bulk pieces: taps 0-1 (64KB, sync ring) gate the
            # first LDWEIGHTS; taps 2-4 land in parallel on the scalar ring;
            # taps 5-8 follow on sync. Ring DMA instructions cost ~600ns to
            # issue regardless of size, so few large transfers beat many
            # small ones.
            # Startup transfer order: the first matmul gates on w taps 0-1
            # plus pair-0 block-0 input, so those go first on their rings;
            # the bulky tap 5-8 piece queues on sync *behind* chunk0a.
            w_sb = wpool.tile([128, KH * KW * C_OUT], MM_DT)
            CH0 = (ROW_BLOCK + KH - 1) * W  # 640 cols
            x0_sb = xpool.tile([128, H * W], MM_DT, tag="x")
            CH0A = 384
            nc.sync.dma_start(x0_sb[:, 0:CH0A], x_d[0:128, 0:CH0A])
            nc.gpsimd.dma_start(x0_sb[:, CH0A:CH0], x_d[0:128, CH0A:CH0])
            nc.sync.dma_start(w_sb[:, 0:2 * C_OUT], w_d[:, 0:2 * C_OUT])
            nc.scalar.dma_start(w_sb[:, 2 * C_OUT:5 * C_OUT],
                                w_d[:, 2 * C_OUT:5 * C_OUT])
            nc.sync.dma_start(w_sb[:, 5 * C_OUT:], w_d[:, 5 * C_OUT:])
            bias_sb = wpool.tile([C_OUT, 1], F32)
            nc.scalar.dma_start(bias_sb[:, :], b_d[:, :])

            for p in range(PAIRS):
                # image pair p: image 2p on partitions 0-63, 2p+1 on 64-127
                # Column-chunks on the gpsimd SWDGE ring: inputs don't queue
                # behind output stores, and sub-tile deps let each row-block
                # start as soon as its chunk lands. Pair 0's first chunk was
                # issued above (trimmed to block-0's rows) so the stream
                # starts as early as possible.
                if p == 0:
                    x_sb = x0_sb
                    bounds = [CH0, 1664, 2688, H * W]
                else:
                    x_sb = xpool.tile([128, H * W], MM_DT, tag="x")
                    bounds = [ch * (H * W // 4) for ch in range(5)]
                for c0, c1 in zip(bounds[:-1], bounds[1:]):
                    nc.gpsimd.dma_start(x_sb[:, c0:c1],
                                        x_d[p * 128:(p + 1) * 128, c0:c1])
                xv = x_sb.rearrange("p (r c) -> p r c", c=W)
                for (R, nr) in ROW_BLOCKS:
                    n_free = nr * WO
                    ps = [psum.tile([C_OUT, n_free], F32, tag="ps",
                                    name=f"ps{p}_{R}_{h}")
                          for h in range(2)]
                    for t in range(KH * KW):
                        kh, kw = divmod(t, KW)
                        for h in (0, 1):
                            rhs = xv[64 * h:64 * h + 64,
                                     R + kh:R + kh + nr, kw:kw + WO]
                            lhsT = w_sb[64 * h:64 * h + 64,
                                        t * C_OUT:(t + 1) * C_OUT]
                            nc.tensor.matmul(
                                ps[h][:, :],
                                lhsT,
                                rhs,
                                start=(t == 0),
                                stop=(t == KH * KW - 1),
                                tile_position=(64 * h, 0),
                            )
                    # PSUM eviction + bias + fp16 downconvert, split across
                    # two engines so neither becomes the pipeline straggler.
                    for h in (0, 1):
                        b_img = p * 2 + h
                        o_sb = opool.tile([C_OUT, n_free], OUT_DT, tag="o")
                        if h == 0:
                            nc.scalar.activation(
                                o_sb[:, :], ps[h][:, :],
                                mybir.ActivationFunctionType.Identity,
                                bias=bias_sb[:, :])
                        else:
                            nc.vector.tensor_scalar_add(
                                o_sb[:, :], ps[h][:, :], bias_sb[:, :])
                        # very last store goes out on the (idle) scalar ring
                        # so the two final slices drain in parallel
                        last = (p == PAIRS - 1 and R == ROW_BLOCKS[-1][0]
                                and h == 1)
                        ring = nc.scalar if last else nc.sync
                        ring.dma_start(
                            y_d[b_img * C_OUT:(b_img + 1) * C_OUT,
                                R * WO:R * WO + n_free],
                            o_sb[:, :])
    nc.compile()
    return nc


def _get_nc():
    if "nc" not in _CACHE:
        _CACHE["nc"] = _build()
    return _CACHE["nc"]


def _make_in_maps(features, weight, bias):
    features = np.asarray(features, dtype=np.float32)
    weight = np.asarray(weight, dtype=np.float32)
    bias = np.asarray(bias, dtype=np.float32)
    in_maps = []
    for c in range(N_CORES):
        e, half = divmod(c, 2)
        b0 = half * B_PER_CORE
        x = np.ascontiguousarray(features[e, b0:b0 + B_PER_CORE]).reshape(
            B_PER_CORE * C_IN, H * W)
        # w[i, (kh*KW+kw)*C_OUT + j] = weight[e, i, j, kh, kw]; duplicated on
        # partitions 64-127 for the upper-row-half matmuls.
        wp = weight[e].transpose(0, 2, 3, 1).reshape(C_IN, KH * KW * C_OUT)
        wp = np.ascontiguousarray(np.concatenate([wp, wp], axis=0))
        in_maps.append({
            "x": x.astype(np.float16),
            "w": wp.astype(np.float16),
            "bias": np.ascontiguousarray(bias[e].reshape(C_OUT, 1)),
        })
    return in_maps


def _assemble(results):
    out = np.empty((E, B, C_OUT, HO, WO), dtype=np.float32)
    for c in range(N_CORES):
        e, half = divmod(c, 2)
        b0 = half * B_PER_CORE
        out[e, b0:b0 + B_PER_CORE] = results[c]["y"].astype(
            np.float32).reshape(B_PER_CORE, C_OUT, HO, WO)
    return out


def kernel(features, weight, bias):
    nc = _get_nc()
    in_maps = _make_in_maps(features, weight, bias)
    res = run_bass_kernel_spmd(nc, in_maps, core_ids=list(range(N_CORES)))
    return _assemble(res.results)


# revision 17
# speedup vs baseline: 1.0426x; 1.0426x over previous
"""Trainium2 Bass kernel for ensemble Conv2d (VALID, 3x3).

Problem: out[e,b,j,y,x] = sum_{i,kh,kw} features[e,b,i,y+kh,x+kw] * weight[e,i,j,kh,kw] + bias[e,j]
  features: (4, 32, 64, 64, 64) f32, weight: (4, 64, 128, 3, 3) f32, bias: (4, 128) f32
  output:   (4, 32, 128, 62, 62) f32

Sharding: E*B = 128 images over 8 cores -> each core handles one ensemble
member e = core//2 and 16 of its 32 images. No cross-core communication.

Per-core kernel: implicit-GEMM convolution. For each image, each 3x3 tap
(kh,kw) is one matmul contraction over C_in=64:
  psum[j, (y,x)] += W[:,j,kh,kw].T @ X[:, y+kh, x+kw]
fp16 matmuls (1 col/cycle, fp32 PSUM accumulation; rel err ~3e-4). Two
images are processed concurrently on the two PE-array row halves
(tile_position (0,0) and (64,0)) so the K=64 contraction doesn't waste half
the array; their weight copies live on the matching partition halves.

Schedule notes (from NTFF traces):
- steady-state MM pair slot is ~211ns (496 cols @2.4GHz + stagger) --
  LDWEIGHTS is hidden by the PE reorder window, so the stream is already
  near the PE floor; the losses were head (weight DMA queued behind
  ACT_TABLE_LOAD on the scalar ring), eviction backlog (all 128 PSUM
  evictions on the one scalar engine -> start-matmuls stall on PSUM
  recycle and a ~13us tail drains after the last MM), and f32 stores.
- fixes: weights load via sync+vector rings, evictions alternate
  scalar/vector engines, outputs stored as fp16 (upconverted on host).
"""

import ml_dtypes
import numpy as np

import concourse.bass as bass
import concourse.mybir as mybir
import concourse.tile as tile
from concourse import bacc
from concourse.bass_utils import run_bass_kernel_spmd

E, B, C_IN, H, W = 4, 32, 64, 64, 64
C_OUT, KH, KW = 128, 3, 3
HO, WO = H - KH + 1, W - KW + 1  # 62, 62
N_CORES = 8
B_PER_CORE = (E * B) // N_CORES  # 16
PAIRS = B_PER_CORE // 2  # 8
ROW_BLOCK = 8  # output rows per matmul block: N = 8*62 = 496 <= 512 (one PSUM bank)
ROW_BLOCKS = [(r * ROW_BLOCK, min(ROW_BLOCK, HO - r * ROW_BLOCK))
              for r in range((HO + ROW_BLOCK - 1) // ROW_BLOCK)]
F32 = mybir.dt.float32
BF16 = mybir.dt.bfloat16
FP16 = mybir.dt.float16

MM_DT = FP16
OUT_DT = FP16  # fp16 stores halve output HBM traffic; host upconverts

_CACHE: dict = {}


def _build():
    nc = bacc.Bacc("TRN2", target_bir_lowering=False, debug=False,
                   num_devices=N_CORES)
    x_d = nc.dram_tensor("x", [B_PER_CORE * C_IN, H * W], MM_DT,
                         kind="ExternalInput").ap()
    w_d = nc.dram_tensor("w", [128, KH * KW * C_OUT], MM_DT,
                         kind="ExternalInput").ap()
    b_d = nc.dram_tensor("bias", [C_OUT, 1], F32, kind="ExternalInput").ap()
    y_d = nc.dram_tensor("y", [B_PER_CORE * C_OUT, HO * WO], OUT_DT,
                         kind="ExternalOutput").ap()

    with tile.TileContext(nc) as tc:
        with (
            tc.tile_pool(name="wpool", bufs=1) as wpool,
            tc.tile_pool(name="xpool", bufs=8) as xpool,
            # deep opool: a store's completion semaphore lands ~2.5us after
            # the store issues, so evictions need >4 output bufs per engine
            # path or they stall on store completions (and start-matmuls on
            # PSUM recycle behind them).
            tc.tile_pool(name="opool", bufs=16) as opool,
            tc.tile_pool(name="psum", bufs=8, space=bass.MemorySpace.PSUM) as psum,
        ):
            # HAM warm-up: the PE clock-gate opens only after ~3.4us of
            # sustained PE activity. Input DMAs gate the real stream until
            # ~11us, so burn the wait on dummy matmuls over a memset tile --
            # the real stream then starts at the full 2.4 GHz.
            warm_sb = wpool.tile([128, 512], MM_DT)
            nc.vector.memset(warm_sb[:, :], 0)
            # borrow a main-pool PSUM bank; it recycles into the block
            # pipeline well after the warm-up matmuls retire
            ps_warm = psum.tile([128, 496], F32, tag="ps", name="ps_warm")
            for _ in range(8):
                nc.tensor.matmul(ps_warm[:, :], warm_sb[:, 0:128],
                                 warm_sb[:, 0:496], start=True, stop=True)
            # Weights in 3 bulk pieces: taps 0-1 (64KB, sync ring) gate the
            # first LDWEIGHTS; taps 2-4 land in parallel on the scalar ring;
            # taps 5-8 follow on sync. Ring DMA instructions cost ~600ns to
            # issue regardless of size, so few large transfers beat many
            # small ones.
            # Startup transfer order: the first matmul gates on w taps 0-1
            # plus pair-0 block-0 input, so those go first on their rings;
            # the bulky tap 5-8 piece queues on sync *behind* chunk0a.
            w_sb = wpool.tile([128, KH * KW * C_OUT], MM_DT)
            CH0 = (ROW_BLOCK + KH - 1) * W  # 640 cols
            x0_sb = xpool.tile([128, H * W], MM_DT, tag="x")
            CH0A = 384
            nc.sync.dma_start(x0_sb[:, 0:CH0A], x_d[0:128, 0:CH0A])
            nc.gpsimd.dma_start(x0_sb[:, CH0A:CH0], x_d[0:128, CH0A:CH0])
            nc.sync.dma_start(w_sb[:, 0:2 * C_OUT], w_d[:, 0:2 * C_OUT])
            nc.scalar.dma_start(w_sb[:, 2 * C_OUT:5 * C_OUT],
                                w_d[:, 2 * C_OUT:5 * C_OUT])
            nc.sync.dma_start(w_sb[:, 5 * C_OUT:7 * C_OUT],
                              w_d[:, 5 * C_OUT:7 * C_OUT])
            nc.scalar.dma_start(w_sb[:, 7 * C_OUT:], w_d[:, 7 * C_OUT:])
            bias_sb = wpool.tile([C_OUT, 1], F32)
            nc.scalar.dma_start(bias_sb[:, :], b_d[:, :])

            for p in range(PAIRS):
                # image pair p: image 2p on partitions 0-63, 2p+1 on 64-127
                # Column-chunks on the gpsimd SWDGE ring: inputs don't queue
                # behind output stores, and sub-tile deps let each row-block
                # start as soon as its chunk lands. Pair 0's first chunk was
                # issued above (trimmed to block-0's rows) so the stream
                # starts as early as possible.
                if p == 0:
                    x_sb = x0_sb
                    bounds = [CH0, 1664, 2688, H * W]
                else:
                    x_sb = xpool.tile([128, H * W], MM_DT, tag="x")
                    bounds = [ch * (H * W // 4) for ch in range(5)]
                for c0, c1 in zip(bounds[:-1], bounds[1:]):
                    nc.gpsimd.dma_start(x_sb[:, c0:c1],
                                        x_d[p * 128:(p + 1) * 128, c0:c1])
                xv = x_sb.rearrange("p (r c) -> p r c", c=W)
                for (R, nr) in ROW_BLOCKS:
                    n_free = nr * WO
                    ps = [psum.tile([C_OUT, n_free], F32, tag="ps",
                                    name=f"ps{p}_{R}_{h}")
                          for h in range(2)]
                    for t in range(KH * KW):
                        kh, kw = divmod(t, KW)
                        for h in (0, 1):
                            rhs = xv[64 * h:64 * h + 64,
                                     R + kh:R + kh + nr, kw:kw + WO]
                            lhsT = w_sb[64 * h:64 * h + 64,
                                        t * C_OUT:(t + 1) * C_OUT]
                            nc.tensor.matmul(
                                ps[h][:, :],
                                lhsT,
                                rhs,
                                start=(t == 0),
                                stop=(t == KH * KW - 1),
                                tile_position=(64 * h, 0),
                            )
                    # PSUM eviction + bias + fp16 downconvert, split across
                    # two engines so neither becomes the pipeline straggler.
                    for h in (0, 1):
                        b_img = p * 2 + h
                        o_sb = opool.tile([C_OUT, n_free], OUT_DT, tag="o")
                        if h == 0:
                            nc.scalar.activation(
                                o_sb[:, :], ps[h][:, :],
                                mybir.ActivationFunctionType.Identity,
                                bias=bias_sb[:, :])
                        else:
                            nc.vector.tensor_scalar_add(
                                o_sb[:, :], ps[h][:, :], bias_sb[:, :])
                        # stores split across two rings: a single ring at
                        # ~600ns/store instruction would be 100% utilized and
                        # its completion jitter stalls the eviction chain
                        ring = nc.scalar if h == 1 else nc.sync
                        ring.dma_start(
                            y_d[b_img * C_OUT:(b_img + 1) * C_OUT,
                                R * WO:R * WO + n_free],
                            o_sb[:, :])
    nc.compile()
    return nc


def _get_nc():
    if "nc" not in _CACHE:
        _CACHE["nc"] = _build()
    return _CACHE["nc"]


def _make_in_maps(features, weight, bias):
    features = np.asarray(features, dtype=np.float32)
    weight = np.asarray(weight, dtype=np.float32)
    bias = np.asarray(bias, dtype=np.float32)
    in_maps = []
    for c in range(N_CORES):
        e, half = divmod(c, 2)
        b0 = half * B_PER_CORE
        x = np.ascontiguousarray(features[e, b0:b0 + B_PER_CORE]).reshape(
            B_PER_CORE * C_IN, H * W)
        # w[i, (kh*KW+kw)*C_OUT + j] = weight[e, i, j, kh, kw]; duplicated on
        # partitions 64-127 for the upper-row-half matmuls.
        wp = weight[e].transpose(0, 2, 3, 1).reshape(C_IN, KH * KW * C_OUT)
        wp = np.ascontiguousarray(np.concatenate([wp, wp], axis=0))
        in_maps.append({
            "x": x.astype(np.float16),
            "w": wp.astype(np.float16),
            "bias": np.ascontiguousarray(bias[e].reshape(C_OUT, 1)),
        })
    return in_maps


def _assemble(results):
    out = np.empty((E, B, C_OUT, HO, WO), dtype=np.float32)
    for c in range(N_CORES):
        e, half = divmod(c, 2)
        b0 = half * B_PER_CORE
        out[e, b0:b0 + B_PER_CORE] = results[c]["y"].astype(
            np.float32).reshape(B_PER_CORE, C_OUT, HO, WO)
    return out


def kernel(features, weight, bias):
    nc = _get_nc()
    in_maps = _make_in_maps(features, weight, bias)
    res = run_bass_kernel_spmd(nc, in_maps, core_ids=list(range(N_CORES)))
    return _assemble(res.results)


# revision 18
# speedup vs baseline: 1.0511x; 1.0082x over previous
"""Trainium2 Bass kernel for ensemble Conv2d (VALID, 3x3).

Problem: out[e,b,j,y,x] = sum_{i,kh,kw} features[e,b,i,y+kh,x+kw] * weight[e,i,j,kh,kw] + bias[e,j]
  features: (4, 32, 64, 64, 64) f32, weight: (4, 64, 128, 3, 3) f32, bias: (4, 128) f32
  output:   (4, 32, 128, 62, 62) f32

Sharding: E*B = 128 images over 8 cores -> each core handles one ensemble
member e = core//2 and 16 of its 32 images. No cross-core communication.

Per-core kernel: implicit-GEMM convolution. For each image, each 3x3 tap
(kh,kw) is one matmul contraction over C_in=64:
  psum[j, (y,x)] += W[:,j,kh,kw].T @ X[:, y+kh, x+kw]
fp16 matmuls (1 col/cycle, fp32 PSUM accumulation; rel err ~3e-4). Two
images are processed concurrently on the two PE-array row halves
(tile_position (0,0) and (64,0)) so the K=64 contraction doesn't waste half
the array; their weight copies live on the matching partition halves.

Schedule notes (from NTFF traces):
- steady-state MM pair slot is ~211ns (496 cols @2.4GHz + stagger) --
  LDWEIGHTS is hidden by the PE reorder window, so the stream is already
  near the PE floor; the losses were head (weight DMA queued behind
  ACT_TABLE_LOAD on the scalar ring), eviction backlog (all 128 PSUM
  evictions on the one scalar engine -> start-matmuls stall on PSUM
  recycle and a ~13us tail drains after the last MM), and f32 stores.
- fixes: weights load via sync+vector rings, evictions alternate
  scalar/vector engines, outputs stored as fp16 (upconverted on host).
"""

import ml_dtypes
import numpy as np

import concourse.bass as bass
import concourse.mybir as mybir
import concourse.tile as tile
from concourse import bacc
from concourse.bass_utils import run_bass_kernel_spmd

E, B, C_IN, H, W = 4, 32, 64, 64, 64
C_OUT, KH, KW = 128, 3, 3
HO, WO = H - KH + 1, W - KW + 1  # 62, 62
N_CORES = 8
B_PER_CORE = (E * B) // N_CORES  # 16
PAIRS = B_PER_CORE // 2  # 8
ROW_BLOCK = 8  # output rows per matmul block: N = 8*62 = 496 <= 512 (one PSUM bank)
ROW_BLOCKS = [(r * ROW_BLOCK, min(ROW_BLOCK, HO - r * ROW_BLOCK))
              for r in range((HO + ROW_BLOCK - 1) // ROW_BLOCK)]
F32 = mybir.dt.float32
BF16 = mybir.dt.bfloat16
FP16 = mybir.dt.float16

MM_DT = FP16
OUT_DT = FP16  # fp16 stores halve output HBM traffic; host upconverts

_CACHE: dict = {}


def _build():
    nc = bacc.Bacc("TRN2", target_bir_lowering=False, debug=False,
                   num_devices=N_CORES)
    x_d = nc.dram_tensor("x", [B_PER_CORE * C_IN, H * W], MM_DT,
                         kind="ExternalInput").ap()
    w_d = nc.dram_tensor("w", [128, KH * KW * C_OUT], MM_DT,
                         kind="ExternalInput").ap()
    b_d = nc.dram_tensor("bias", [C_OUT, 1], F32, kind="ExternalInput").ap()
    y_d = nc.dram_tensor("y", [B_PER_CORE * C_OUT, HO * WO], OUT_DT,
                         kind="ExternalOutput").ap()

    with tile.TileContext(nc) as tc:
        with (
            tc.tile_pool(name="wpool", bufs=1) as wpool,
            tc.tile_pool(name="xpool", bufs=8) as xpool,
            # deep opool: a store's completion semaphore lands ~2.5us after
            # the store issues, so evictions need >4 output bufs per engine
            # path or they stall on store completions (and start-matmuls on
            # PSUM recycle behind them).
            tc.tile_pool(name="opool", bufs=16) as opool,
            tc.tile_pool(name="psum", bufs=8, space=bass.MemorySpace.PSUM) as psum,
        ):
            # HAM warm-up: the PE clock-gate opens only after ~3.4us of
            # sustained PE activity. Input DMAs gate the real stream until
            # ~11us, so burn the wait on dummy matmuls over a memset tile --
            # the real stream then starts at the full 2.4 GHz.
            warm_sb = wpool.tile([128, 512], MM_DT)
            nc.vector.memset(warm_sb[:, :], 0)
            # borrow a main-pool PSUM bank; it recycles into the block
            # pipeline well after the warm-up matmuls retire
            ps_warm = psum.tile([128, 496], F32, tag="ps", name="ps_warm")
            for _ in range(8):
                nc.tensor.matmul(ps_warm[:, :], warm_sb[:, 0:128],
                                 warm_sb[:, 0:496], start=True, stop=True)
            # Weights in 3 bulk pieces: taps 0-1 (64KB, sync ring) gate the
            # first LDWEIGHTS; taps 2-4 land in parallel on the scalar ring;
            # taps 5-8 follow on sync. Ring DMA instructions cost ~600ns to
            # issue regardless of size, so few large transfers beat many
            # small ones.
            # Startup transfer order: the first matmul gates on w taps 0-1
            # plus pair-0 block-0 input, so those go first on their rings;
            # the bulky tap 5-8 piece queues on sync *behind* chunk0a.
            w_sb = wpool.tile([128, KH * KW * C_OUT], MM_DT)
            CH0 = (ROW_BLOCK + KH - 1) * W  # 640 cols
            x0_sb = xpool.tile([128, H * W], MM_DT, tag="x")
            CH0A = 384
            nc.sync.dma_start(x0_sb[:, 0:CH0A], x_d[0:128, 0:CH0A])
            nc.gpsimd.dma_start(x0_sb[:, CH0A:CH0], x_d[0:128, CH0A:CH0])
            nc.sync.dma_start(w_sb[:, 0:2 * C_OUT], w_d[:, 0:2 * C_OUT])
            # remaining taps paired by deadline, alternating rings so each
            # piece's completion semaphore beats the tap's first matmul
            nc.scalar.dma_start(w_sb[:, 2 * C_OUT:4 * C_OUT],
                                w_d[:, 2 * C_OUT:4 * C_OUT])
            nc.sync.dma_start(w_sb[:, 4 * C_OUT:6 * C_OUT],
                              w_d[:, 4 * C_OUT:6 * C_OUT])
            nc.scalar.dma_start(w_sb[:, 6 * C_OUT:], w_d[:, 6 * C_OUT:])
            bias_sb = wpool.tile([C_OUT, 1], F32)
            nc.scalar.dma_start(bias_sb[:, :], b_d[:, :])

            for p in range(PAIRS):
                # image pair p: image 2p on partitions 0-63, 2p+1 on 64-127
                # Column-chunks on the gpsimd SWDGE ring: inputs don't queue
                # behind output stores, and sub-tile deps let each row-block
                # start as soon as its chunk lands. Pair 0's first chunk was
                # issued above (trimmed to block-0's rows) so the stream
                # starts as early as possible.
                if p == 0:
                    x_sb = x0_sb
                    bounds = [CH0, 1664, 2688, H * W]
                else:
                    x_sb = xpool.tile([128, H * W], MM_DT, tag="x")
                    bounds = [ch * (H * W // 4) for ch in range(5)]
                for c0, c1 in zip(bounds[:-1], bounds[1:]):
                    nc.gpsimd.dma_start(x_sb[:, c0:c1],
                                        x_d[p * 128:(p + 1) * 128, c0:c1])
                xv = x_sb.rearrange("p (r c) -> p r c", c=W)
                for (R, nr) in ROW_BLOCKS:
                    n_free = nr * WO
                    ps = [psum.tile([C_OUT, n_free], F32, tag="ps",
                                    name=f"ps{p}_{R}_{h}")
                          for h in range(2)]
                    for t in range(KH * KW):
                        kh, kw = divmod(t, KW)
                        for h in (0, 1):
                            rhs = xv[64 * h:64 * h + 64,
                                     R + kh:R + kh + nr, kw:kw + WO]
                            lhsT = w_sb[64 * h:64 * h + 64,
                                        t * C_OUT:(t + 1) * C_OUT]
                            nc.tensor.matmul(
                                ps[h][:, :],
                                lhsT,
                                rhs,
                                start=(t == 0),
                                stop=(t == KH * KW - 1),
                                tile_position=(64 * h, 0),
                            )
                    # PSUM eviction + bias + fp16 downconvert, split across
                    # two engines so neither becomes the pipeline straggler.
                    for h in (0, 1):
                        b_img = p * 2 + h
                        o_sb = opool.tile([C_OUT, n_free], OUT_DT, tag="o")
                        if h == 0:
                            nc.scalar.activation(
                                o_sb[:, :], ps[h][:, :],
                                mybir.ActivationFunctionType.Identity,
                                bias=bias_sb[:, :])
                        else:
                            nc.vector.tensor_scalar_add(
                                o_sb[:, :], ps[h][:, :], bias_sb[:, :])
                        # stores split across two rings: a single ring at
                        # ~600ns/store instruction would be 100% utilized and
                        # its completion jitter stalls the eviction chain
                        ring = nc.scalar if h == 1 else nc.sync
                        ring.dma_start(
                            y_d[b_img * C_OUT:(b_img + 1) * C_OUT,
                                R * WO:R * WO + n_free],
                            o_sb[:, :])
    nc.compile()
    return nc


def _get_nc():
    if "nc" not in _CACHE:
        _CACHE["nc"] = _build()
    return _CACHE["nc"]


def _make_in_maps(features, weight, bias):
    features = np.asarray(features, dtype=np.float32)
    weight = np.asarray(weight, dtype=np.float32)
    bias = np.asarray(bias, dtype=np.float32)
    in_maps = []
    for c in range(N_CORES):
        e, half = divmod(c, 2)
        b0 = half * B_PER_CORE
        x = np.ascontiguousarray(features[e, b0:b0 + B_PER_CORE]).reshape(
            B_PER_CORE * C_IN, H * W)
        # w[i, (kh*KW+kw)*C_OUT + j] = weight[e, i, j, kh, kw]; duplicated on
        # partitions 64-127 for the upper-row-half matmuls.
        wp = weight[e].transpose(0, 2, 3, 1).reshape(C_IN, KH * KW * C_OUT)
        wp = np.ascontiguousarray(np.concatenate([wp, wp], axis=0))
        in_maps.append({
            "x": x.astype(np.float16),
            "w": wp.astype(np.float16),
            "bias": np.ascontiguousarray(bias[e].reshape(C_OUT, 1)),
        })
    return in_maps


def _assemble(results):
    out = np.empty((E, B, C_OUT, HO, WO), dtype=np.float32)
    for c in range(N_CORES):
        e, half = divmod(c, 2)
        b0 = half * B_PER_CORE
        out[e, b0:b0 + B_PER_CORE] = results[c]["y"].astype(
            np.float32).reshape(B_PER_CORE, C_OUT, HO, WO)
    return out


def kernel(features, weight, bias):
    nc = _get_nc()
    in_maps = _make_in_maps(features, weight, bias)
    res = run_bass_kernel_spmd(nc, in_maps, core_ids=list(range(N_CORES)))
    return _assemble(res.results)
